# revision 1
# baseline (speedup 1.0000x reference)
"""Trainium2 Bass kernel for nn_BGAN (GNN message passing), 8 NeuronCores.

Node-sharded SPMD with replicated weights:
  A. z-phase: z = h @ W_fc.T plus fused per-node scalars (e_src, zw0, zw1,
     e_dst, hw0) in one matmul; rows written to a pair table (two 640B node
     rows per 1280B pair, so dma_gather's int16 index = node>>1 fits).
  B. deg histogram via iota/is_equal one-hots + PE matmul grid; AllReduce.
  C. AllGather pair table + hw0; hw = hw0*rsqrt(deg) written into the local
     copy of the gathered table (strided column DMA).
  D. mailbox: dma_gather K=10 pair rows per node (host parity masks pick the
     real half); attention softmax; row conv from gathered scalars; col conv
     via per-k diagonal matmuls accumulated in PSUM; updatefeat matmul.
  E. GraphConv agg -> group softmax weights -> weighted mean folded into the
     final matmul -> AllGather partials -> classifier.

kernel(**inputs): FULL numpy inputs -> FULL [1, C] output.
"""
import sys
import types

import numpy as np

sys.path.insert(0, "/opt/trn_rl_repo")

import concourse.bass as bass
import concourse.bacc as bacc
import concourse.mybir as mybir
import concourse.tile as tile
from concourse import bass_utils
from concourse.bass import broadcast_tensor_aps
from concourse.masks import make_identity
from concourse.tile import add_dep_helper

P = 128
D = 256
K = 10
C_CLS = 40
NCORES = 8
EPS = 1e-5

NODE_F32 = 160           # 640B node row
PAIR_F32 = 2 * NODE_F32
SC_ESRC = 128
SC_ZW0 = 129
SC_ZW1 = 130
SC_HW = 131

F32 = mybir.dt.float32
BF16 = mybir.dt.bfloat16
I32 = mybir.dt.int32
I16 = mybir.dt.int16
AF = mybir.ActivationFunctionType
ALU = mybir.AluOpType
AX = mybir.AxisListType


def _ntff_hook():
    try:
        import antenv
        from trn_agent_boot.trn_boot import _ntff_profile_via_ctypes
        mod = types.ModuleType("antenv.axon_hooks")
        _state = {"hook": None}
        mod.set_axon_ntff_profile_hook = lambda h: _state.update(hook=h)
        mod.get_axon_ntff_profile_hook = lambda: _state["hook"]
        sys.modules["antenv.axon_hooks"] = mod
        antenv.axon_hooks = mod
        mod.set_axon_ntff_profile_hook(
            _ntff_profile_via_ctypes("/opt/axon/libaxon_pjrt.so"))
    except Exception:
        pass


def bc(a, b):
    """broadcast b against a, return broadcasted b."""
    _, b2 = broadcast_tensor_aps(a, b)
    return b2


def build(n_nodes, scal, debug=False, stop_after=None):
    NLOC = n_nodes // NCORES
    NT = NLOC // P
    NCH = NLOC // 512
    EL = NLOC * K
    HCH = EL // P
    NPAIR = n_nodes // 2
    NG = NLOC // 256
    NB = n_nodes // P
    HIR = n_nodes // 256          # used hi-rows of the deg grid
    MCH = (HIR + P - 1) // P

    nc = bacc.Bacc("TRN2", num_devices=NCORES, dynamic_dma_scratch_size=65536)
    rg = [list(range(NCORES))]

    h_in = nc.dram_tensor("h", [NLOC, D], F32, kind="ExternalInput")
    wfc = nc.dram_tensor("wfc", [D, D], F32, kind="ExternalInput")
    wfct = nc.dram_tensor("wfct", [D, D], F32, kind="ExternalInput")
    awp = nc.dram_tensor("awp", [D, 4], F32, kind="ExternalInput")
    wgc = nc.dram_tensor("wgc", [D, 1], F32, kind="ExternalInput")
    wcol_i = nc.dram_tensor("wcol", [1, K], F32, kind="ExternalInput")
    lw = nc.dram_tensor("lw", [K - 1 + D, D], BF16, kind="ExternalInput")
    wcls = nc.dram_tensor("wcls", [D, C_CLS], F32, kind="ExternalInput")
    bcls = nc.dram_tensor("bcls", [1, C_CLS], F32, kind="ExternalInput")
    widx = nc.dram_tensor("widx", [NCH, P, 40], I32, kind="ExternalInput")
    hi_i = nc.dram_tensor("hi", [P, HCH], F32, kind="ExternalInput")
    lo_i = nc.dram_tensor("lo", [P, HCH], F32, kind="ExternalInput")

    out_t = nc.dram_tensor("out", [1, C_CLS], F32, kind="ExternalOutput")
    dbg = {}
    if debug:
        dbg["deg"] = nc.dram_tensor("dbg_deg", [P, NB], F32, kind="ExternalOutput")
        dbg["agg"] = nc.dram_tensor("dbg_agg", [NT, P], F32, kind="ExternalOutput")
        dbg["w"] = nc.dram_tensor("dbg_w", [NT, P], F32, kind="ExternalOutput")
        dbg["uf"] = nc.dram_tensor("dbg_uf", [P, D], F32, kind="ExternalOutput")
        dbg["col0"] = nc.dram_tensor("dbg_col0", [P, D], F32, kind="ExternalOutput")
        dbg["alpha"] = nc.dram_tensor("dbg_alpha", [P, 40], F32, kind="ExternalOutput")
        dbg["hg"] = nc.dram_tensor("dbg_hg", [1, D], F32, kind="ExternalOutput")
        dbg["zrow"] = nc.dram_tensor("dbg_zrow", [P, NODE_F32], F32, kind="ExternalOutput")

    zp_sh = nc.dram_tensor("zp_sh", [NLOC, NODE_F32], F32, kind="Internal")
    zp_full = nc.dram_tensor("zp_full", [n_nodes, NODE_F32], F32, kind="Internal")
    deg_part = nc.dram_tensor("deg_part", [n_nodes], F32, kind="Internal")
    deg_full = nc.dram_tensor("deg_full", [n_nodes], F32, kind="Internal",
                              addr_space="Shared")
    hw0_part = nc.dram_tensor("hw0_part", [NLOC], F32, kind="Internal")
    hw0_full = nc.dram_tensor("hw0_full", [n_nodes], F32, kind="Internal",
                              addr_space="Shared")
    agg_d = nc.dram_tensor("agg_d", [NT, P], F32, kind="Internal")
    hgp_part = nc.dram_tensor("hgp_part", [1, D], F32, kind="Internal")
    hgp_full = nc.dram_tensor("hgp_full", [NCORES, D], F32, kind="Internal",
                              addr_space="Shared")

    with tile.TileContext(nc) as tc:
        with tc.tile_pool(name="cst", bufs=1) as cst, \
             tc.tile_pool(name="sb", bufs=2) as sb, \
             tc.tile_pool(name="res", bufs=1) as res:

            ident = cst.tile([P, P], F32)
            make_identity(nc, ident[:, :])
            identb = cst.tile([P, P], BF16)
            nc.vector.tensor_copy(out=identb[:, :], in_=ident[:, :])

            wfc_sb = cst.tile([P, 2, D], F32)
            nc.sync.dma_start(out=wfc_sb[:, 0, :], in_=wfc[0:P, :])
            nc.sync.dma_start(out=wfc_sb[:, 1, :], in_=wfc[P:D, :])
            awp_sb = cst.tile([P, 2, 4], F32)
            nc.sync.dma_start(out=awp_sb[:, 0, :], in_=awp[0:P, :])
            nc.sync.dma_start(out=awp_sb[:, 1, :], in_=awp[P:D, :])
            wcol_rep = cst.tile([P, K], F32)
            nc.sync.dma_start(out=wcol_rep[:, :],
                              in_=wcol_i[0:1, :].to_broadcast([P, K]))
            lw_sb = cst.tile([P, 3, D], BF16)
            nc.vector.memset(lw_sb[:, 0, :], 0.0)
            nc.sync.dma_start(out=lw_sb[0:K - 1, 0, :], in_=lw[0:K - 1, :])
            nc.sync.dma_start(out=lw_sb[:, 1, :], in_=lw[K - 1:K - 1 + P, :])
            nc.sync.dma_start(out=lw_sb[:, 2, :], in_=lw[K - 1 + P:K - 1 + D, :])

            edst_res = res.tile([P, NT], F32)
            hw0_res = res.tile([P, NT], F32)
            ufr = res.tile([P, NT * D], BF16)
            agg_all = res.tile([P, NT], F32)

            # ================= phase A + B (own psum pool) =================
            with tc.tile_pool(name="psa", bufs=2, space="PSUM") as psa, \
                 tc.tile_pool(name="psg", bufs=1, space="PSUM") as psg:
                v_ps = psa.tile([P, 2, 4], F32, space="PSUM", tag="v_ps")
                for m in range(2):
                    for pp in range(2):
                        nc.tensor.matmul(
                            out=v_ps[:, m, :],
                            lhsT=wfc_sb[:, pp, m * P:(m + 1) * P],
                            rhs=awp_sb[:, pp, :],
                            start=(pp == 0), stop=(pp == 1))
                rext = cst.tile([P, 2, 261], F32)
                for hh in range(2):
                    nc.sync.dma_start(out=rext[:, hh, 0:D],
                                      in_=wfct[hh * P:(hh + 1) * P, :])
                    nc.scalar.copy(out=rext[:, hh, 256:257], in_=v_ps[:, hh, 0:1])
                    nc.scalar.copy(out=rext[:, hh, 257:258], in_=v_ps[:, hh, 2:3])
                    nc.scalar.copy(out=rext[:, hh, 258:259], in_=v_ps[:, hh, 3:4])
                    nc.scalar.copy(out=rext[:, hh, 259:260], in_=v_ps[:, hh, 1:2])
                    nc.sync.dma_start(out=rext[:, hh, 260:261],
                                      in_=wgc[hh * P:(hh + 1) * P, :])

                zp_wr = []
                for t in range(NT):
                    h_t = sb.tile([P, D], F32, tag="h_t")
                    nc.sync.dma_start(out=h_t[:, :], in_=h_in[t * P:(t + 1) * P, :])
                    hT_ps = psa.tile([P, 2, P], F32, space="PSUM", tag="hT_ps")
                    nc.tensor.transpose(out=hT_ps[:, 0, :], in_=h_t[:, 0:P],
                                        identity=ident[:, :])
                    nc.tensor.transpose(out=hT_ps[:, 1, :], in_=h_t[:, P:D],
                                        identity=ident[:, :])
                    hT = sb.tile([P, 2, P], F32, tag="hT")
                    nc.scalar.copy(out=hT[:, 0, :], in_=hT_ps[:, 0, :])
                    nc.scalar.copy(out=hT[:, 1, :], in_=hT_ps[:, 1, :])
                    zx = psa.tile([P, 261], F32, space="PSUM", tag="zx")
                    nc.tensor.matmul(out=zx[:, :], lhsT=hT[:, 0, :], rhs=rext[:, 0, :],
                                     start=True, stop=False)
                    nc.tensor.matmul(out=zx[:, :], lhsT=hT[:, 1, :], rhs=rext[:, 1, :],
                                     start=False, stop=True)
                    stg = sb.tile([P, NODE_F32], F32, tag="stg")
                    nc.vector.memset(stg[:, SC_HW:NODE_F32], 0.0)
                    stgb = stg[:, :].bitcast(BF16)
                    nc.scalar.activation(out=stgb[:, 0:D], in_=zx[:, 0:D], func=AF.Copy)
                    nc.vector.tensor_copy(out=stg[:, SC_ESRC:SC_ESRC + 3],
                                          in_=zx[:, 256:259])
                    nc.vector.tensor_copy(out=edst_res[:, t:t + 1], in_=zx[:, 259:260])
                    nc.vector.tensor_copy(out=hw0_res[:, t:t + 1], in_=zx[:, 260:261])
                    w1 = nc.sync.dma_start(out=zp_sh[t * P:(t + 1) * P, :], in_=stg[:, :])
                    zp_wr.append(w1)
                hw0s = sb.tile([P, NT], F32, tag="hw0s")
                nc.vector.tensor_copy(out=hw0s[:, :], in_=hw0_res[:, :])
                w2 = nc.sync.dma_start(
                    out=hw0_part.rearrange("(t p) -> p t", p=P), in_=hw0s[:, :])

                # ---- B. histogram ----
                iota_i = cst.tile([P, D], I32)
                nc.gpsimd.iota(iota_i[:, :], pattern=[[1, D]], base=0,
                               channel_multiplier=0)
                iota_b = cst.tile([P, D], BF16)
                nc.vector.tensor_copy(out=iota_b[:, :], in_=iota_i[:, :])
                hi_sb = res.tile([P, HCH], F32)
                lo_sb = res.tile([P, HCH], F32)
                nc.sync.dma_start(out=hi_sb[:, :], in_=hi_i[:, :])
                nc.sync.dma_start(out=lo_sb[:, :], in_=lo_i[:, :])
                grid = psg.tile([P, MCH, D], F32, space="PSUM")
                for ch in range(HCH):
                    ohh = sb.tile([P, D], BF16, tag="ohh")
                    ohl = sb.tile([P, D], BF16, tag="ohl")
                    nc.vector.tensor_scalar(out=ohh[:, :], in0=iota_b[:, :],
                                            scalar1=hi_sb[:, ch:ch + 1], scalar2=None,
                                            op0=ALU.is_equal)
                    nc.vector.tensor_scalar(out=ohl[:, :], in0=iota_b[:, :],
                                            scalar1=lo_sb[:, ch:ch + 1], scalar2=None,
                                            op0=ALU.is_equal)
                    for m in range(MCH):
                        rows = min(P, HIR - m * P)
                        nc.tensor.matmul(out=grid[0:rows, m, :],
                                         lhsT=ohh[:, m * P:m * P + rows],
                                         rhs=ohl[:, :],
                                         start=(ch == 0), stop=(ch == HCH - 1))
                dwr = []
                for m in range(MCH):
                    rows = min(P, HIR - m * P)
                    gsb = sb.tile([P, D], F32, tag="gsb")
                    nc.scalar.copy(out=gsb[0:rows, :], in_=grid[0:rows, m, :])
                    dwr.append(nc.sync.dma_start(
                        out=deg_part.rearrange("(m p c) -> m p c", p=P, c=D)[m, 0:rows, :]
                        if MCH > 1 else
                        deg_part.rearrange("(p c) -> p c", c=D)[m * P:m * P + rows, :],
                        in_=gsb[0:rows, :]))

            cc_deg = nc.gpsimd.collective_compute(
                "AllReduce", ALU.add, ins=[deg_part[:]], outs=[deg_full[:]],
                replica_groups=rg)
            for w in dwr:
                add_dep_helper(cc_deg.ins, w.ins, True, "deg AR after writes")
            cc_zp = nc.gpsimd.collective_compute(
                "AllGather", ALU.bypass, ins=[zp_sh[:, :]], outs=[zp_full[:, :]],
                replica_groups=rg)
            for w in zp_wr:
                add_dep_helper(cc_zp.ins, w.ins, True, "zp AG after writes")
            cc_hw0 = nc.gpsimd.collective_compute(
                "AllGather", ALU.bypass, ins=[hw0_part[:]], outs=[hw0_full[:]],
                replica_groups=rg)
            add_dep_helper(cc_hw0.ins, w2.ins, True, "hw0 AG after write")

            # ---- C. hw column ----
            hw_wr = []
            BSZ = 128 if NB % 128 == 0 else NB
            nblk = max(1, NB // BSZ)
            for blk in range(nblk):
                bsz = min(BSZ, NB)
                dg = sb.tile([P, bsz], F32, tag="dg")
                r1 = nc.sync.dma_start(
                    out=dg[:, :],
                    in_=deg_full.rearrange("(p b) -> p b", p=P)[:, blk * BSZ:blk * BSZ + bsz])
                add_dep_helper(r1.ins, cc_deg.ins, True, "deg read after AR")
                h0 = sb.tile([P, bsz], F32, tag="h0")
                r2 = nc.sync.dma_start(
                    out=h0[:, :],
                    in_=hw0_full.rearrange("(p b) -> p b", p=P)[:, blk * BSZ:blk * BSZ + bsz])
                add_dep_helper(r2.ins, cc_hw0.ins, True, "hw0 read after AG")
                mx = sb.tile([P, bsz], F32, tag="mx")
                nc.vector.tensor_scalar(out=mx[:, :], in0=dg[:, :], scalar1=1.0,
                                        scalar2=None, op0=ALU.max)
                rc = sb.tile([P, bsz], F32, tag="rc")
                nc.vector.reciprocal(out=rc[:, :], in_=mx[:, :])
                sq = sb.tile([P, bsz], F32, tag="sq")
                nc.scalar.activation(out=sq[:, :], in_=rc[:, :], func=AF.Sqrt)
                msk = sb.tile([P, bsz], F32, tag="msk")
                nc.vector.tensor_scalar(out=msk[:, :], in0=dg[:, :], scalar1=0.0,
                                        scalar2=None, op0=ALU.is_gt)
                nc.vector.tensor_tensor(out=sq[:, :], in0=sq[:, :], in1=msk[:, :],
                                        op=ALU.mult)
                hwv = sb.tile([P, bsz], F32, tag="hwv")
                nc.vector.tensor_tensor(out=hwv[:, :], in0=h0[:, :], in1=sq[:, :],
                                        op=ALU.mult)
                ww = nc.sync.dma_start(
                    out=zp_full.rearrange("(p b) f -> p b f", p=P)[
                        :, blk * BSZ:blk * BSZ + bsz, SC_HW:SC_HW + 1],
                    in_=hwv[:, :].rearrange("p (b o) -> p b o", o=1))
                add_dep_helper(ww.ins, cc_zp.ins, True, "hw col write after zp AG")
                hw_wr.append(ww)
                if debug:
                    dd = nc.sync.dma_start(out=dbg["deg"][:, blk * BSZ:blk * BSZ + bsz],
                                           in_=dg[:, :])

            # ================= D. mailbox =================
            zp_pair = zp_full.rearrange("(q two) f -> q (two f)", two=2)
            if stop_after == "C" or stop_after == "CD_SENTINEL":
                oo0 = sb.tile([1, C_CLS], F32, tag="oo0")
                nc.vector.memset(oo0[:, :], 0.0)
                nc.vector.tensor_scalar(out=oo0[:, :], in0=oo0[:, :],
                                        scalar1=hw_wr and 0.0 or 0.0, scalar2=None,
                                        op0=ALU.add)
                od = nc.sync.dma_start(out=out_t[:, :], in_=oo0[:, :])
                for ww in hw_wr:
                    add_dep_helper(od.ins, ww.ins, True, "out after hw col")
                return_early = True
            else:
                return_early = False
            if return_early:
                pass
            else:
             with tc.tile_pool(name="mailp", bufs=2) as mailp, \
                 tc.tile_pool(name="psm", bufs=2, space="PSUM") as psm:
                for chh in range(NCH):
                    mail = mailp.tile([P, 40, NODE_F32], F32, tag="mail")
                    wsb = sb.tile([P, 40], I32, tag="wsb")
                    nc.sync.dma_start(out=wsb[:, :], in_=widx[chh, :, :])
                    g0 = None
                    for cc_ in range(40):
                        g = nc.gpsimd.indirect_dma_start(
                            out=mail[:, cc_, :], out_offset=None, in_=zp_full[:, :],
                            in_offset=bass.IndirectOffsetOnAxis(
                                ap=wsb[:, cc_:cc_ + 1], axis=0))
                        if g0 is None:
                            g0 = g
                            add_dep_helper(g.ins, cc_zp.ins, True, "gather after zp AG")
                            for ww in hw_wr:
                                add_dep_helper(g.ins, ww.ins, True, "gather after hw col")
                        else:
                            add_dep_helper(g.ins, g0.ins, True, "gather chain")

                    esr = mail[:, :, SC_ESRC]
                    zw0 = mail[:, :, SC_ZW0]
                    zw1 = mail[:, :, SC_ZW1]
                    hwg = mail[:, :, SC_HW]

                    ee = sb.tile([P, 40], F32, tag="ee")
                    e3 = ee[:, :].rearrange("p (k j) -> p k j", k=K)
                    ed3 = edst_res[:, chh * 4:(chh + 1) * 4].rearrange(
                        "p (o j) -> p o j", o=1)
                    nc.vector.tensor_tensor(
                        out=e3, in0=esr.rearrange("p (k j) -> p k j", k=K),
                        in1=bc(e3, ed3), op=ALU.add)
                    eesc = sb.tile([P, 40], F32, tag="eesc")
                    nc.vector.tensor_scalar(out=eesc[:, :], in0=ee[:, :],
                                            scalar1=0.01, scalar2=None, op0=ALU.mult)
                    nc.vector.tensor_tensor(out=ee[:, :], in0=ee[:, :],
                                            in1=eesc[:, :], op=ALU.max)
                    emax = sb.tile([P, 4], F32, tag="emax")
                    nc.vector.tensor_reduce(
                        out=emax[:, :], in_=ee[:, :].rearrange("p (k j) -> p j k", k=K),
                        axis=AX.X, op=ALU.max)
                    es = sb.tile([P, 40], F32, tag="es")
                    es3 = es[:, :].rearrange("p (k j) -> p k j", k=K)
                    nc.vector.tensor_tensor(
                        out=es3, in0=e3,
                        in1=bc(es3, emax[:, :].rearrange("p (o j) -> p o j", o=1)),
                        op=ALU.subtract)
                    ex = sb.tile([P, 40], F32, tag="ex")
                    nc.scalar.activation(out=ex[:, :], in_=es[:, :], func=AF.Exp)
                    esum = sb.tile([P, 4], F32, tag="esum")
                    nc.vector.tensor_reduce(
                        out=esum[:, :], in_=ex[:, :].rearrange("p (k j) -> p j k", k=K),
                        axis=AX.X, op=ALU.add)
                    erec = sb.tile([P, 4], F32, tag="erec")
                    nc.vector.reciprocal(out=erec[:, :], in_=esum[:, :])
                    alp = sb.tile([P, 40], F32, tag="alp")
                    al3 = alp[:, :].rearrange("p (k j) -> p k j", k=K)
                    nc.vector.tensor_tensor(
                        out=al3, in0=ex[:, :].rearrange("p (k j) -> p k j", k=K),
                        in1=bc(al3, erec[:, :].rearrange("p (o j) -> p o j", o=1)),
                        op=ALU.mult)

                    bet = sb.tile([P, 40], F32, tag="bet")
                    b3 = bet[:, :].rearrange("p (k j) -> p k j", k=K)
                    nc.vector.tensor_tensor(
                        out=b3, in0=al3,
                        in1=bc(b3, wcol_rep[:, :].rearrange("p (k o) -> p k o", o=1)),
                        op=ALU.mult)


                    agt = sb.tile([P, 4], F32, tag="agt")
                    nc.vector.tensor_reduce(
                        out=agt[:, :], in_=hwg.rearrange("p (k j) -> p j k", k=K),
                        axis=AX.X, op=ALU.add)
                    nc.vector.tensor_copy(out=agg_all[:, chh * 4:(chh + 1) * 4],
                                          in_=agt[:, :])

                    if stop_after == "D1":
                        continue
                    r0 = sb.tile([P, 40], F32, tag="r0")
                    r1_ = sb.tile([P, 40], F32, tag="r1_")
                    nc.vector.tensor_tensor(out=r0[:, :], in0=alp[:, :], in1=zw0,
                                            op=ALU.mult)
                    nc.vector.tensor_tensor(out=r1_[:, :], in0=alp[:, :], in1=zw1,
                                            op=ALU.mult)
                    rowp = sb.tile([P, 4, 16], F32, tag="rowp")
                    nc.vector.memset(rowp[:, :, K - 1:16], 0.0)
                    nc.vector.tensor_tensor(
                        out=rowp[:, :, 0:K - 1].rearrange("p j k -> p k j"),
                        in0=r0[:, :].rearrange("p (k j) -> p k j", k=K)[:, 0:K - 1, :],
                        in1=r1_[:, :].rearrange("p (k j) -> p k j", k=K)[:, 1:K, :],
                        op=ALU.add)
                    nc.scalar.activation(out=rowp[:, :, 0:K - 1],
                                         in_=rowp[:, :, 0:K - 1], func=AF.Relu,
                                         bias=scal["bias_row"], scale=scal["s_row"])

                    mailb = mail[:, :, :].bitcast(BF16)
                    for jj in range(4):
                        t = chh * 4 + jj
                        colp = psm.tile([P, D], F32, space="PSUM", tag="colp")
                        for k in range(K):
                            cidx = k * 4 + jj
                            dg_ = sb.tile([P, P], BF16, tag="diag")
                            nc.vector.tensor_scalar(
                                out=dg_[:, :], in0=identb[:, :],
                                scalar1=bet[:, cidx:cidx + 1], scalar2=None,
                                op0=ALU.mult)
                            nc.tensor.matmul(
                                out=colp[:, :], lhsT=dg_[:, :],
                                rhs=mailb[:, cidx, 0:D],
                                start=(k == 0), stop=(k == K - 1))
                        colr = sb.tile([P, D], BF16, tag="colr")
                        nc.scalar.activation(out=colr[:, :], in_=colp[:, :],
                                             func=AF.Relu, bias=scal["bias_col"],
                                             scale=scal["s_col"])
                        if debug and t == 0:
                            cdbg = sb.tile([P, D], F32, tag="cdbg")
                            nc.vector.tensor_copy(out=cdbg[:, :], in_=colr[:, :])
                            nc.sync.dma_start(out=dbg["col0"][:, :], in_=cdbg[:, :])
                            nc.sync.dma_start(out=dbg["alpha"][:, :], in_=alp[:, :])
                        if stop_after == "D2":
                            continue
                        ctp = psm.tile([P, 2, P], BF16, space="PSUM", tag="ctp")
                        nc.tensor.transpose(out=ctp[:, 0, :], in_=colr[:, 0:P],
                                            identity=identb[:, :])
                        nc.tensor.transpose(out=ctp[:, 1, :], in_=colr[:, P:D],
                                            identity=identb[:, :])
                        colT = sb.tile([P, 2, P], BF16, tag="colT")
                        nc.scalar.copy(out=colT[:, 0, :], in_=ctp[:, 0, :])
                        nc.scalar.copy(out=colT[:, 1, :], in_=ctp[:, 1, :])
                        rtp = psm.tile([16, P], F32, space="PSUM", tag="rtp")
                        nc.tensor.transpose(out=rtp[:, :], in_=rowp[:, jj, :],
                                            identity=ident[:, :])
                        rowT = sb.tile([16, P], BF16, tag="rowT")
                        nc.scalar.copy(out=rowT[:, :], in_=rtp[:, :])
                        if stop_after == "D3":
                            continue
                        ufp = psm.tile([P, D], F32, space="PSUM", tag="ufp")
                        nc.tensor.matmul(out=ufp[:, :], lhsT=rowT[0:K - 1, :],
                                         rhs=lw_sb[0:K - 1, 0, :], start=True, stop=False)
                        nc.tensor.matmul(out=ufp[:, :], lhsT=colT[:, 0, :],
                                         rhs=lw_sb[:, 1, :], start=False, stop=False)
                        nc.tensor.matmul(out=ufp[:, :], lhsT=colT[:, 1, :],
                                         rhs=lw_sb[:, 2, :], start=False, stop=True)
                        h_t2 = sb.tile([P, D], F32, tag="h_t2")
                        nc.sync.dma_start(out=h_t2[:, :], in_=h_in[t * P:(t + 1) * P, :])
                        ufs = sb.tile([P, D], F32, tag="ufs")
                        nc.vector.tensor_tensor(out=ufs[:, :], in0=ufp[:, :],
                                                in1=h_t2[:, :], op=ALU.add)
                        nc.scalar.activation(out=ufr[:, t * D:(t + 1) * D],
                                             in_=ufs[:, :], func=AF.Relu)
                        if debug and t == 0:
                            nc.scalar.activation(out=ufs[:, :], in_=ufs[:, :],
                                                 func=AF.Relu)
                            nc.sync.dma_start(out=dbg["uf"][:, :], in_=ufs[:, :])

            # ================= E. weights + final =================
            if stop_after is not None:
                return nc
            with tc.tile_pool(name="pse", bufs=1, space="PSUM") as pse:
                ag2 = sb.tile([P, NT], F32, tag="ag2")
                nc.vector.tensor_scalar(out=ag2[:, :], in0=agg_all[:, :],
                                        scalar1=scal["n_dst"], scalar2=scal["b_gc"],
                                        op0=ALU.mult, op1=ALU.add)
                aw = nc.sync.dma_start(out=agg_d.rearrange("t p -> p t"), in_=ag2[:, :])
                if debug:
                    ad = nc.sync.dma_start(out=dbg["agg"][:, :], in_=agg_d[:, :])
                    add_dep_helper(ad.ins, aw.ins, True, "dbg agg")
                asm = sb.tile([NG, 256], F32, tag="asm")
                ar = nc.sync.dma_start(
                    out=asm[:, :], in_=agg_d.rearrange("(g a) p -> g (a p)", a=2))
                add_dep_helper(ar.ins, aw.ins, True, "agg read after write")
                amx = sb.tile([NG, 1], F32, tag="amx")
                nc.vector.tensor_reduce(out=amx[:, :], in_=asm[:, :], axis=AX.X,
                                        op=ALU.max)
                nc.vector.tensor_scalar(out=asm[:, :], in0=asm[:, :],
                                        scalar1=amx[:, 0:1], scalar2=None,
                                        op0=ALU.subtract)
                aex = sb.tile([NG, 256], F32, tag="aex")
                asum = sb.tile([NG, 1], F32, tag="asum")
                nc.scalar.activation(out=aex[:, :], in_=asm[:, :], func=AF.Exp,
                                     accum_out=asum[:, :])
                arec = sb.tile([NG, 1], F32, tag="arec")
                nc.vector.reciprocal(out=arec[:, :], in_=asum[:, :])
                wgt = sb.tile([NG, 256], BF16, tag="wgt")
                nc.vector.tensor_scalar(out=wgt[:, :], in0=aex[:, :],
                                        scalar1=arec[:, 0:1], scalar2=scal["inv_n"],
                                        op0=ALU.mult, op1=ALU.mult)
                wtp = pse.tile([P, 2, NG], BF16, space="PSUM", tag="wtp")
                nc.tensor.transpose(out=wtp[:, 0, 0:NG], in_=wgt[:, 0:P],
                                    identity=identb[0:NG, 0:NG])
                nc.tensor.transpose(out=wtp[:, 1, 0:NG], in_=wgt[:, P:256],
                                    identity=identb[0:NG, 0:NG])
                wT = sb.tile([P, NT], BF16, tag="wT")
                wTv = wT[:, :].rearrange("p (g a) -> p g a", a=2)
                nc.scalar.copy(out=wTv[:, :, 0], in_=wtp[:, 0, 0:NG])
                nc.scalar.copy(out=wTv[:, :, 1], in_=wtp[:, 1, 0:NG])
                if debug:
                    wdbg = sb.tile([P, NT], F32, tag="wdbg")
                    nc.vector.tensor_copy(out=wdbg[:, :], in_=wT[:, :])
                    nc.sync.dma_start(out=dbg["w"].rearrange("t p -> p t"),
                                      in_=wdbg[:, :])

                hgp0 = pse.tile([P, 1], F32, space="PSUM", tag="hgp0")
                hgp1 = pse.tile([P, 1], F32, space="PSUM", tag="hgp1")
                hgps = [hgp0, hgp1]
                for t in range(NT):
                    for m in range(2):
                        nc.tensor.matmul(
                            out=hgps[m][:, :],
                            lhsT=ufr[:, t * D + m * P:t * D + (m + 1) * P],
                            rhs=wT[:, t:t + 1], start=(t == 0), stop=(t == NT - 1))
                hgs = sb.tile([P, 2], F32, tag="hgs")
                nc.vector.tensor_copy(out=hgs[:, 0:1], in_=hgps[0][:, :])
                nc.vector.tensor_copy(out=hgs[:, 1:2], in_=hgps[1][:, :])
                hw3 = nc.sync.dma_start(
                    out=hgp_part.rearrange("o (m p) -> p (o m)", p=P), in_=hgs[:, :])
                cc_hg = nc.gpsimd.collective_compute(
                    "AllGather", ALU.bypass, ins=[hgp_part[:, :]], outs=[hgp_full[:, :]],
                    replica_groups=rg)
                add_dep_helper(cc_hg.ins, hw3.ins, True, "hg AG after write")
                hgf = sb.tile([P, 2, NCORES], F32, tag="hgf")
                for m in range(2):
                    hr = nc.sync.dma_start(
                        out=hgf[:, m, :],
                        in_=hgp_full[:, m * P:(m + 1) * P].rearrange("c p -> p c"))
                    add_dep_helper(hr.ins, cc_hg.ins, True, "hg read after AG")
                hg = sb.tile([P, 2], F32, tag="hg")
                nc.vector.tensor_reduce(
                    out=hg[:, :], in_=hgf[:, :, :], axis=AX.X, op=ALU.add)
                if debug:
                    nc.sync.dma_start(
                        out=dbg["hg"].rearrange("o (m p) -> p (o m)", p=P),
                        in_=hg[:, :])
                wcls_sb = sb.tile([P, 2, C_CLS], F32, tag="wcls_sb")
                nc.sync.dma_start(out=wcls_sb[:, 0, :], in_=wcls[0:P, :])
                nc.sync.dma_start(out=wcls_sb[:, 1, :], in_=wcls[P:D, :])
                outp = pse.tile([1, C_CLS], F32, space="PSUM", tag="outp")
                for m in range(2):
                    nc.tensor.matmul(out=outp[:, :], lhsT=hg[:, m:m + 1],
                                     rhs=wcls_sb[:, m, :], start=(m == 0), stop=(m == 1))
                bcl = sb.tile([1, C_CLS], F32, tag="bcl")
                nc.sync.dma_start(out=bcl[:, :], in_=bcls[:, :])
                oo = sb.tile([1, C_CLS], F32, tag="oo")
                nc.vector.tensor_tensor(out=oo[:, :], in0=outp[:, :], in1=bcl[:, :],
                                        op=ALU.add)
                nc.sync.dma_start(out=out_t[:, :], in_=oo[:, :])

    return nc


def prep_inputs(h, neighbors, W_fc, a_attn, w_row, b_row, g_row, be_row,
                w_col, b_col, g_col, be_col, localw, W_gc, b_gc, W_cls, b_cls):
    import ml_dtypes
    h = np.asarray(h)
    n_nodes = h.shape[0]
    NLOC = n_nodes // NCORES
    NCH = NLOC // 512
    HCH = NLOC * K // P
    nb = np.asarray(neighbors).astype(np.int64)
    a_attn = np.asarray(a_attn)
    w_row = np.asarray(w_row)

    s_row = float(np.float32(np.asarray(g_row)[0]) / np.sqrt(np.float32(1.0 + EPS)))
    s_col = float(np.float32(np.asarray(g_col)[0]) / np.sqrt(np.float32(1.0 + EPS)))
    scal = dict(
        s_row=s_row,
        bias_row=float(np.float32(np.asarray(b_row)[0]) * np.float32(s_row)
                       + np.float32(np.asarray(be_row)[0])),
        s_col=s_col,
        bias_col=float(np.float32(np.asarray(b_col)[0]) * np.float32(s_col)
                       + np.float32(np.asarray(be_col)[0])),
        n_dst=float(1.0 / np.sqrt(np.float32(K))),
        b_gc=float(np.asarray(b_gc)[0]),
        inv_n=float(np.float32(1.0) / np.float32(n_nodes)),
    )

    awp = np.stack([a_attn[:D], a_attn[D:], w_row[0], w_row[1]],
                   axis=1).astype(np.float32)
    common = {
        "wfc": np.ascontiguousarray(np.asarray(W_fc).astype(np.float32)),
        "wfct": np.ascontiguousarray(np.asarray(W_fc).T.astype(np.float32)),
        "awp": awp,
        "wgc": np.asarray(W_gc).astype(np.float32).reshape(D, 1),
        "wcol": np.asarray(w_col).astype(np.float32).reshape(1, K),
        "lw": np.asarray(localw).astype(ml_dtypes.bfloat16),
        "wcls": np.asarray(W_cls).astype(np.float32),
        "bcls": np.asarray(b_cls).astype(np.float32).reshape(1, C_CLS),
    }

    in_maps = []
    for c in range(NCORES):
        nbl = nb[c * NLOC:(c + 1) * NLOC]
        widx = np.zeros((NCH, P, 40), np.int32)
        for ch in range(NCH):
            blk = nbl[ch * 512:(ch + 1) * 512]
            for k in range(K):
                for jj in range(4):
                    widx[ch, :, k * 4 + jj] = blk[jj * P:(jj + 1) * P, k]
        hil = nbl.reshape(-1)
        hi = (hil >> 8).astype(np.float32).reshape(HCH, P).T.copy()
        lo = (hil & 255).astype(np.float32).reshape(HCH, P).T.copy()
        m = {
            "h": np.ascontiguousarray(h[c * NLOC:(c + 1) * NLOC].astype(np.float32)),
            "widx": widx,
            "hi": np.ascontiguousarray(hi),
            "lo": np.ascontiguousarray(lo),
        }
        m.update(common)
        in_maps.append(m)
    return in_maps, scal, n_nodes


_CACHE = {}


def run(inputs, debug=False, trace=False):
    _ntff_hook()
    in_maps, scal, n_nodes = prep_inputs(**inputs)
    key = (n_nodes, tuple(sorted(scal.items())), debug)
    if key not in _CACHE:
        nc = build(n_nodes, scal, debug=debug)
        nc.finalize()
        _CACHE[key] = nc
    nc = _CACHE[key]
    return bass_utils.run_bass_kernel_spmd(
        nc, in_maps, core_ids=list(range(NCORES)), trace=trace)


def kernel(**inputs):
    res = run(inputs, debug=False, trace=False)
    return np.asarray(res.results[0]["out"], dtype=np.float32)

